# revision 7
# baseline (speedup 1.0000x reference)
"""BiLSTM-CRF on 8 trn2 NeuronCores (Bass/Tile), self-contained.

Strategy (sharding_hint: data-parallel over sentences; batch=1 here so):
- Host: embedding gather + weight layout prep (input marshalling only).
- Phase A (all 8 cores): input projections x@W_ih^T+b for this core's T/8
  token slice, both directions; AllGather the per-step gate preactivations.
- Phase B: single SPMD recurrence; per-core DATA selects the role:
  core 0 consumes the forward stream + W_hh_f, core 1 the time-reversed
  stream + W_hh_b, cores 2-7 zeros. Outputs write masked (mF/mB) into a
  shared hs buffer -> one AllReduce makes hf/hb visible everywhere.
- Phase C: every core computes CRF emission feats for all chunks (cheap),
  mask-selects its own chunk.
- Phase V: chunked Viterbi. V1 basis max-plus scan per chunk; V2 tiny
  AllGather + redundant boundary combine; V3 true-init scan emitting
  one-hot backpointer matrices; V3b chunk pointer-map composition; V2b
  cross-chunk tag chain; V4 within-chunk backtrace. All per-tag algebra is
  5-partition DVE/PE ops.
"""

import numpy as np

import concourse.bass as bass
import concourse.mybir as mybir
import concourse.tile as tile
from concourse import bacc
from concourse.bass_utils import run_bass_kernel_spmd
from concourse.masks import make_identity

F32 = mybir.dt.float32
I32 = mybir.dt.int32
AF = mybir.ActivationFunctionType
ALU = mybir.AluOpType
AX = mybir.AxisListType

NCORE = 8
NTAG = 5
START, STOP = 3, 4
NEG = -10000.0
H2 = 512
EMB = 1024
NM = 16          # gate row tiles (2048/128)
NK = 4           # h contraction tiles (512/128)
NKE = 8          # emb contraction tiles (1024/128)

# gate permutation: m-tiles [i(0:4) f(4:8) o(8:12) g(12:16)]
# pytorch order in weights: i, f, g, o
def _gate_perm():
    return np.concatenate([np.arange(0, 512), np.arange(512, 1024),
                           np.arange(1536, 2048), np.arange(1024, 1536)])


def build_program(T, TBLK):
    TC = T // NCORE          # chunk (timesteps per core)
    CBLK = TC // TBLK        # blocks per chunk
    NBLK = T // TBLK         # global blocks
    assert T % NCORE == 0 and TC % TBLK == 0 and TBLK <= 128
    assert TC <= 512, "phase-A matmul N and psum bank assume TC<=512"

    nc = bacc.Bacc("TRN2", target_bir_lowering=False, debug=False,
                   enable_asserts=False, num_devices=NCORE)

    # ---------------- I/O ----------------
    xT_in = nc.dram_tensor("xT_in", [2, NKE, 128, TC], F32, kind="ExternalInput")
    wihT_in = nc.dram_tensor("wihT_in", [2, NKE, NM, 128, 128], F32, kind="ExternalInput")
    bias_in = nc.dram_tensor("bias_in", [128, 2 * NM], F32, kind="ExternalInput")
    whh_in = nc.dram_tensor("whh_in", [NM * NK, 128, 128], F32, kind="ExternalInput")
    h0c0_in = nc.dram_tensor("h0c0_in", [128, 8], F32, kind="ExternalInput")
    masks_in = nc.dram_tensor("masks_in", [128, 2], F32, kind="ExternalInput")
    woutT_in = nc.dram_tensor("woutT_in", [NKE, 128, NTAG], F32, kind="ExternalInput")
    bout_in = nc.dram_tensor("bout_in", [1, NTAG], F32, kind="ExternalInput")
    trans_in = nc.dram_tensor("trans_in", [NTAG, NTAG], F32, kind="ExternalInput")
    transrep_in = nc.dram_tensor("transrep_in", [NTAG, NTAG * NTAG], F32, kind="ExternalInput")
    transstop_in = nc.dram_tensor("transstop_in", [1, NTAG], F32, kind="ExternalInput")
    dinit_in = nc.dram_tensor("dinit_in", [NTAG, NTAG], F32, kind="ExternalInput")
    fv0_in = nc.dram_tensor("fv0_in", [1, NTAG], F32, kind="ExternalInput")
    iota5_in = nc.dram_tensor("iota5_in", [NTAG, 1], F32, kind="ExternalInput")
    cmaskrow_in = nc.dram_tensor("cmaskrow_in", [1, NCORE * NTAG], F32, kind="ExternalInput")
    cmask5_in = nc.dram_tensor("cmask5_in", [NTAG, NCORE], F32, kind="ExternalInput")

    score_out = nc.dram_tensor("score_out", [1, 1], F32, kind="ExternalOutput")
    path_out = nc.dram_tensor("path_out", [CBLK, TBLK], I32, kind="ExternalOutput")

    # internal DRAM
    pre_loc = nc.dram_tensor("pre_loc", [CBLK, 2, NM, 128, TBLK], F32)
    pre_gath = nc.dram_tensor("pre_gath", [NCORE, CBLK, 2, NM, 128, TBLK], F32,
                              addr_space="Shared")
    hs_loc = nc.dram_tensor("hs_loc", [2, NBLK, H2, TBLK], F32)
    hs_all = nc.dram_tensor("hs_all", [2, NBLK, H2, TBLK], F32, addr_space="Shared")
    m_loc = nc.dram_tensor("m_loc", [NTAG, NTAG], F32)
    m_gath = nc.dram_tensor("m_gath", [NCORE, NTAG, NTAG], F32, addr_space="Shared")
    f_loc = nc.dram_tensor("f_loc", [NTAG, NTAG], F32)
    f_gath = nc.dram_tensor("f_gath", [NCORE, NTAG, NTAG], F32, addr_space="Shared")

    RG = [list(range(NCORE))]

    with tile.TileContext(nc) as tc:
        with tc.tile_pool(name="persist", bufs=1) as pp:
            ident = pp.tile([128, 128], F32)
            make_identity(nc, ident)
            masks = pp.tile([128, 2], F32)
            nc.gpsimd.dma_start(masks[:], masks_in[:])
            mF = masks[:, 0:1]
            mB = masks[:, 1:2]

            # ================= Phase A: input projections =================
            with (
                tc.tile_pool(name="pa_sb", bufs=2) as asb,
                tc.tile_pool(name="pa_w", bufs=4) as awp,
                tc.tile_pool(name="pa_ps", bufs=2, space="PSUM") as aps,
            ):
                xt = asb.tile([128, 2 * NKE * TC], F32, tag="xt")
                for d in range(2):
                    for k in range(NKE):
                        nc.sync.dma_start(
                            xt[:, (d * NKE + k) * TC:(d * NKE + k + 1) * TC],
                            xT_in[d, k])
                bias_sb = asb.tile([128, 2 * NM], F32, tag="bias")
                nc.gpsimd.dma_start(bias_sb[:], bias_in[:])

                for d in range(2):
                    for m in range(NM):
                        pacc = aps.tile([128, TC], F32)
                        for k in range(NKE):
                            wk = awp.tile([128, 128], F32, tag="wk")
                            nc.sync.dma_start(wk[:], wihT_in[d, k, m])
                            nc.tensor.matmul(
                                pacc[:],
                                wk[:],
                                xt[:, (d * NKE + k) * TC:(d * NKE + k + 1) * TC],
                                start=(k == 0), stop=(k == NKE - 1))
                        po = asb.tile([128, TC], F32, tag="po")
                        nc.vector.tensor_scalar_add(
                            po[:], pacc[:], bias_sb[:, d * NM + m:d * NM + m + 1])
                        for b in range(CBLK):
                            nc.sync.dma_start(
                                pre_loc[b, d, m, :, :],
                                po[:, b * TBLK:(b + 1) * TBLK])

            nc.gpsimd.collective_compute(
                "AllGather", ALU.bypass, replica_groups=RG,
                ins=[pre_loc[:]], outs=[pre_gath[:]])

            # ================= Phase B: LSTM recurrence =================
            with (
                tc.tile_pool(name="pb_sb", bufs=1) as bsb,
                tc.tile_pool(name="pb_roll", bufs=3) as brl,
                tc.tile_pool(name="pb_ps", bufs=2, space="PSUM") as bps,
            ):
                wh = bsb.tile([128, NM * NK * 128], F32)
                nc.sync.dma_start(
                    wh[:].rearrange("p (n q) -> p n q", n=NM * NK),
                    whh_in[:].rearrange("n p q -> p n q"))
                hc = bsb.tile([128, 8], F32)
                nc.gpsimd.dma_start(hc[:], h0c0_in[:])
                h = hc[:, 0:4]
                c = hc[:, 4:8]
                pa_f = bsb.tile([128, NM * TBLK], F32)
                pa_b = bsb.tile([128, NM * TBLK], F32)
                hstage = bsb.tile([128, 4 * TBLK], F32)
                hmask = bsb.tile([128, 4 * TBLK], F32)

                def body(iB):
                    nc.sync.dma_start(
                        pa_f[:].rearrange("p (m s) -> p m s", m=NM),
                        pre_gath[:].rearrange("a b c d e f -> (a b) c d e f")[
                            bass.ds(iB, 1), 0].squeeze(0).rearrange(
                                "m p s -> p m s"))
                    nc.sync.dma_start(
                        pa_b[:].rearrange("p (m s) -> p m s", m=NM),
                        pre_gath[:].rearrange("a b c d e f -> (a b) c d e f")[
                            bass.ds(iB, 1), 1].squeeze(0).rearrange(
                                "m p s -> p m s"))
                    nc.vector.tensor_scalar_mul(pa_f[:], pa_f[:], mF)
                    nc.vector.tensor_scalar_mul(pa_b[:], pa_b[:], mB)
                    nc.vector.tensor_add(pa_f[:], pa_f[:], pa_b[:])

                    for s in range(TBLK):
                        pg = bps.tile([128, NM], F32, tag="pg")
                        for m in range(NM):
                            for k in range(NK):
                                nc.tensor.matmul(
                                    pg[:, m:m + 1],
                                    wh[:, (m * NK + k) * 128:(m * NK + k + 1) * 128],
                                    h[:, k:k + 1],
                                    start=(k == 0), stop=(k == NK - 1))
                        gates = brl.tile([128, NM], F32, tag="gates")
                        nc.vector.tensor_add(gates[:], pg[:], pa_f[:, s::TBLK])
                        siof = brl.tile([128, 12], F32, tag="siof")
                        nc.scalar.activation(siof[:], gates[:, 0:12], AF.Sigmoid)
                        tg = brl.tile([128, 4], F32, tag="tg")
                        nc.scalar.activation(tg[:], gates[:, 12:16], AF.Tanh)
                        fc = brl.tile([128, 4], F32, tag="fc")
                        nc.vector.tensor_mul(fc[:], siof[:, 4:8], c)
                        ig = brl.tile([128, 4], F32, tag="ig")
                        nc.vector.tensor_mul(ig[:], siof[:, 0:4], tg[:])
                        nc.vector.tensor_add(c, fc[:], ig[:])
                        tc_ = brl.tile([128, 4], F32, tag="tc_")
                        nc.scalar.activation(tc_[:], c, AF.Tanh)
                        nc.vector.tensor_mul(h, siof[:, 8:12], tc_[:])
                        nc.vector.tensor_copy(hstage[:, s::TBLK], h)

                    # masked writes of this block's h rows
                    hsv = hs_loc[:].rearrange("d b e s -> (d b) e s")
                    nc.vector.tensor_scalar_mul(hmask[:], hstage[:], mF)
                    nc.sync.dma_start(
                        hsv[bass.ds(iB, 1)].squeeze(0).rearrange(
                            "(j p) s -> p j s", j=4),
                        hmask[:].rearrange("p (j s) -> p j s", j=4))
                    nc.vector.tensor_scalar_mul(hmask[:], hstage[:], mB)
                    nc.sync.dma_start(
                        hsv[bass.ds(iB + NBLK, 1)].squeeze(0).rearrange(
                            "(j p) s -> p j s", j=4),
                        hmask[:].rearrange("p (j s) -> p j s", j=4))

                with tc.For_i(0, NBLK, hint_engines=(mybir.EngineType.PE,)) as iB:
                    body(iB)

            nc.gpsimd.collective_compute(
                "AllReduce", ALU.add, replica_groups=RG,
                ins=[hs_loc[:]], outs=[hs_all[:]])

            # ================= Phase C: feats =================
            # feats_all[p, (cc, b, tag)] for every chunk cc
            with (
                tc.tile_pool(name="pc_sb", bufs=2) as csb,
                tc.tile_pool(name="pc_ps", bufs=2, space="PSUM") as cps,
                tc.tile_pool(name="pv", bufs=1) as vp,
            ):
                woutT = csb.tile([128, NKE * NTAG], F32, tag="wout")
                nc.sync.dma_start(
                    woutT[:].rearrange("p (k n) -> p k n", k=NKE),
                    woutT_in[:].rearrange("k p n -> p k n"))
                bout = csb.tile([1, NTAG], F32, tag="bout")
                nc.sync.dma_start(bout[:], bout_in[:])
                ones1 = csb.tile([1, 128], F32, tag="ones1")
                nc.vector.memset(ones1[:], 1.0)

                feats_all = vp.tile([128, NCORE * CBLK * NTAG], F32)
                for cc in range(NCORE):
                    for b in range(CBLK):
                        Bg = cc * CBLK + b
                        Br = NBLK - 1 - Bg
                        pf = cps.tile([128, NTAG], F32, tag="vps")
                        for j8 in range(NKE):
                            dirn, j = divmod(j8, 4)
                            lt = csb.tile([128, TBLK], F32, tag="lt")
                            if dirn == 0:
                                nc.sync.dma_start(
                                    lt[:], hs_all[0, Bg, j * 128:(j + 1) * 128, :])
                            else:
                                nc.sync.dma_start(
                                    lt[:], hs_all[1, Br, j * 128:(j + 1) * 128, ::-1])
                            nc.tensor.matmul(
                                pf[:TBLK, :], lt[:], woutT[:, j8 * NTAG:(j8 + 1) * NTAG],
                                start=(j8 == 0), stop=False)
                        nc.tensor.matmul(
                            pf[:TBLK, :], ones1[:, :TBLK], bout[:],
                            start=False, stop=True)
                        nc.vector.tensor_copy(
                            feats_all[:TBLK, (cc * CBLK + b) * NTAG:
                                      (cc * CBLK + b + 1) * NTAG],
                            pf[:TBLK, :])

                # mask-select my chunk's feats
                feats = vp.tile([128, CBLK * NTAG], F32)
                cm5 = csb.tile([NTAG, NCORE], F32, tag="cm5")
                nc.sync.dma_start(cm5[:], cmask5_in[:])
                cmcol = csb.tile([128, NCORE], F32, tag="cmcol")
                nc.sync.dma_start(
                    cmcol[:], cmask5_in[0:1, :].to_broadcast((128, NCORE)))
                nc.vector.memset(feats[:], 0.0)
                for cc in range(NCORE):
                    tmpm = csb.tile([128, CBLK * NTAG], F32, tag="tmpm")
                    nc.vector.tensor_scalar_mul(
                        tmpm[:TBLK, :],
                        feats_all[:TBLK, cc * CBLK * NTAG:(cc + 1) * CBLK * NTAG],
                        cmcol[:TBLK, cc:cc + 1])
                    nc.vector.tensor_add(feats[:TBLK, :], feats[:TBLK, :],
                                         tmpm[:TBLK, :])

                # ============== Phase V: chunked viterbi ==============
                N5 = NTAG * NTAG
                trans_sb = csb.tile([NTAG, NTAG], F32, tag="trans")
                nc.sync.dma_start(trans_sb[:], trans_in[:])
                transrep = csb.tile([NTAG, N5], F32, tag="transrep")
                nc.sync.dma_start(transrep[:], transrep_in[:])
                tstop = csb.tile([1, NTAG], F32, tag="tstop")
                nc.sync.dma_start(tstop[:], transstop_in[:])
                fv0 = csb.tile([1, NTAG], F32, tag="fv0")
                nc.sync.dma_start(fv0[:], fv0_in[:])
                iota5 = csb.tile([NTAG, 1], F32, tag="iota5")
                nc.sync.dma_start(iota5[:], iota5_in[:])
                cmrow = csb.tile([1, NCORE * NTAG], F32, tag="cmrow")
                nc.sync.dma_start(cmrow[:], cmaskrow_in[:])
                ones5 = csb.tile([1, NTAG], F32, tag="ones5")
                nc.vector.memset(ones5[:], 1.0)

                # --- build Arep (trans+feat replicated) and featsT per block ---
                arep = vp.tile([NTAG, CBLK * TBLK * N5], F32)
                featsT = vp.tile([NTAG, CBLK * TBLK], F32)
                TG = 20
                NGR = (TBLK + TG - 1) // TG
                for b in range(CBLK):
                    ftp = cps.tile([NTAG, TBLK], F32, tag="vps")
                    nc.tensor.transpose(
                        ftp[:], feats[:TBLK, b * NTAG:(b + 1) * NTAG],
                        ident[:TBLK, :TBLK])
                    nc.vector.tensor_copy(
                        featsT[:, b * TBLK:(b + 1) * TBLK], ftp[:])
                    frow_sb = csb.tile([1, NTAG * TBLK], F32, tag="frow_sb")
                    for to in range(NTAG):
                        frp = cps.tile([1, TBLK], F32, tag="vps")
                        nc.tensor.matmul(
                            frp[:],
                            ident[:NTAG, to:to + 1],
                            featsT[:, b * TBLK:(b + 1) * TBLK],
                            start=True, stop=True)
                        nc.vector.tensor_copy(
                            frow_sb[:, to * TBLK:(to + 1) * TBLK], frp[:])
                    for g in range(NGR):
                        t0 = g * TG
                        tn = min(TG, TBLK - t0)
                        agp = cps.tile([NTAG, 512], F32, tag="vps")
                        # rhs[0, t, to, j] = feats[t0+t, to]; layout (to, t)
                        rhs = frow_sb[:].rearrange(
                            "p (a t) -> p t a", a=NTAG)[:, t0:t0 + tn, :]
                        rhs = rhs.unsqueeze(3).to_broadcast((1, tn, NTAG, NTAG))
                        nc.tensor.matmul(
                            agp[:, :tn * N5], ones5[:], rhs,
                            start=True, stop=True)
                        dst = arep[:, (b * TBLK + t0) * N5:(b * TBLK + t0 + tn) * N5]
                        nc.vector.tensor_add(
                            dst.rearrange("p (t q) -> p t q", t=tn),
                            agp[:, :tn * N5].rearrange("p (t q) -> p t q", t=tn),
                            transrep[:].unsqueeze(1).to_broadcast((NTAG, tn, N5)))

                # --- V1: basis scan ---
                D = vp.tile([NTAG, NTAG], F32)
                nc.sync.dma_start(D[:], dinit_in[:])
                tmp_s = csb.tile([NTAG, N5], F32, tag="tmp_s")
                for t in range(TC):
                    nc.vector.tensor_add(
                        tmp_s[:].rearrange("p (a j) -> p a j", a=NTAG),
                        D[:].unsqueeze(1).to_broadcast((NTAG, NTAG, NTAG)),
                        arep[:, t * N5:(t + 1) * N5].rearrange(
                            "p (a j) -> p a j", a=NTAG))
                    nc.vector.reduce_max(
                        D[:], tmp_s[:].rearrange("p (a j) -> p a j", a=NTAG),
                        axis=AX.X)
                # contribute M^T
                mtp = cps.tile([NTAG, NTAG], F32, tag="vps")
                nc.tensor.transpose(mtp[:], D[:], ident[:NTAG, :NTAG])
                mts = csb.tile([NTAG, NTAG], F32, tag="mts")
                nc.vector.tensor_copy(mts[:], mtp[:])
                nc.sync.dma_start(m_loc[:], mts[:])

                nc.gpsimd.collective_compute(
                    "AllGather", ALU.bypass, replica_groups=RG,
                    ins=[m_loc[:]], outs=[m_gath[:]])

                # --- V2: boundary combine (redundant on every core) ---
                mall = vp.tile([NTAG, NCORE * NTAG], F32)
                nc.sync.dma_start(
                    mall[:].rearrange("p (c s) -> p c s", c=NCORE),
                    m_gath[:].rearrange("c p s -> p c s"))
                fvrow = vp.tile([1, NTAG], F32)
                nc.vector.tensor_copy(fvrow[:], fv0[:])
                entryrows = vp.tile([1, NCORE * NTAG], F32)
                tmpv = csb.tile([NTAG, NTAG], F32, tag="tmpv")
                fvcol = csb.tile([NTAG, 1], F32, tag="fvcol")
                for cc in range(NCORE):
                    nc.vector.tensor_copy(
                        entryrows[:, cc * NTAG:(cc + 1) * NTAG], fvrow[:])
                    rp = cps.tile([NTAG, NTAG], F32, tag="vps")
                    nc.tensor.matmul(rp[:], ones5[:], fvrow[:],
                                     start=True, stop=True)
                    nc.vector.tensor_add(
                        tmpv[:], rp[:], mall[:, cc * NTAG:(cc + 1) * NTAG])
                    nc.vector.reduce_max(fvcol[:], tmpv[:], axis=AX.X)
                    rt = cps.tile([1, NTAG], F32, tag="vps")
                    nc.tensor.transpose(rt[:], fvcol[:], ident[:NTAG, :NTAG])
                    nc.vector.tensor_copy(fvrow[:], rt[:])
                # terminal / score / best tag
                term = csb.tile([1, NTAG], F32, tag="term")
                nc.vector.tensor_add(term[:], fvrow[:], tstop[:])
                score_sb = csb.tile([1, 1], F32, tag="score")
                nc.vector.reduce_max(score_sb[:], term[:], axis=AX.X)
                nc.sync.dma_start(score_out[:], score_sb[:])
                bestrow = csb.tile([1, NTAG], F32, tag="bestrow")
                nc.vector.tensor_scalar(
                    out=bestrow[:], in0=term[:], scalar1=score_sb[:],
                    scalar2=None, op0=ALU.is_equal)
                bestcol_p = cps.tile([NTAG, 1], F32, tag="vps")
                nc.tensor.transpose(bestcol_p[:], bestrow[:], ident[:1, :1])
                bestcol = vp.tile([NTAG, 1], F32)
                nc.vector.tensor_copy(bestcol[:], bestcol_p[:])
                # my entry row
                entry_m = vp.tile([1, NTAG], F32)
                masked = csb.tile([1, NCORE * NTAG], F32, tag="masked")
                nc.vector.tensor_mul(masked[:], entryrows[:], cmrow[:])
                nc.vector.reduce_max(
                    entry_m[:],
                    masked[:].rearrange("p (c a) -> p a c", c=NCORE),
                    axis=AX.X)

                # --- V3: true scan with backpointer one-hots ---
                D3 = vp.tile([NTAG, NTAG], F32)
                rp0 = cps.tile([NTAG, NTAG], F32, tag="vps")
                nc.tensor.matmul(rp0[:], ones5[:], entry_m[:], start=True, stop=True)
                nc.vector.tensor_copy(D3[:], rp0[:])
                g_all = vp.tile([NTAG, TC * NTAG], F32)
                D3n = vp.tile([NTAG, NTAG], F32)
                for t in range(TC):
                    ntvp = cps.tile([NTAG, NTAG], F32, tag="vps")
                    nc.tensor.matmul(ntvp[:], ones5[:], D3[0:1, :],
                                     start=True, stop=True)
                    ntv = csb.tile([NTAG, NTAG], F32, tag="ntv")
                    nc.vector.tensor_add(ntv[:], ntvp[:], trans_sb[:])
                    mcol = csb.tile([NTAG, 1], F32, tag="mcol")
                    nc.vector.reduce_max(mcol[:], ntv[:], axis=AX.X)
                    nc.vector.tensor_scalar(
                        out=g_all[:, t * NTAG:(t + 1) * NTAG], in0=ntv[:],
                        scalar1=mcol[:], scalar2=None, op0=ALU.is_equal)
                    # scan update
                    nc.vector.tensor_add(
                        tmp_s[:].rearrange("p (a j) -> p a j", a=NTAG),
                        D3[:].unsqueeze(1).to_broadcast((NTAG, NTAG, NTAG)),
                        arep[:, t * N5:(t + 1) * N5].rearrange(
                            "p (a j) -> p a j", a=NTAG))
                    nc.vector.reduce_max(
                        D3n[:], tmp_s[:].rearrange("p (a j) -> p a j", a=NTAG),
                        axis=AX.X)
                    nc.vector.tensor_copy(D3[:], D3n[:])

                # --- V3b: compose chunk pointer map F = G_t0 ... G_e ---
                Fm = vp.tile([NTAG, NTAG], F32)
                nc.vector.tensor_copy(Fm[:], ident[:NTAG, :NTAG])
                for t in range(TC - 1, -1, -1):
                    fp = cps.tile([NTAG, NTAG], F32, tag="vps")
                    nc.tensor.matmul(
                        fp[:], g_all[:, t * NTAG:(t + 1) * NTAG], Fm[:],
                        start=True, stop=True)
                    nc.vector.tensor_copy(Fm[:], fp[:])
                ftp2 = cps.tile([NTAG, NTAG], F32, tag="vps")
                nc.tensor.transpose(ftp2[:], Fm[:], ident[:NTAG, :NTAG])
                fts = csb.tile([NTAG, NTAG], F32, tag="fts")
                nc.vector.tensor_copy(fts[:], ftp2[:])
                nc.sync.dma_start(f_loc[:], fts[:])

                nc.gpsimd.collective_compute(
                    "AllGather", ALU.bypass, replica_groups=RG,
                    ins=[f_loc[:]], outs=[f_gath[:]])

                # --- V2b: cross-chunk last-tag chain ---
                fall = vp.tile([NTAG, NCORE * NTAG], F32)
                nc.sync.dma_start(
                    fall[:].rearrange("p (c s) -> p c s", c=NCORE),
                    f_gath[:].rearrange("c p s -> p c s"))
                lastvs = vp.tile([NTAG, NCORE], F32)
                vcur = vp.tile([NTAG, 1], F32)
                nc.vector.tensor_copy(vcur[:], bestcol[:])
                for cc in range(NCORE - 1, -1, -1):
                    nc.vector.tensor_copy(lastvs[:, cc:cc + 1], vcur[:])
                    if cc > 0:
                        vp_ = cps.tile([NTAG, 1], F32, tag="vps")
                        nc.tensor.matmul(
                            vp_[:], fall[:, cc * NTAG:(cc + 1) * NTAG], vcur[:],
                            start=True, stop=True)
                        nc.vector.tensor_copy(vcur[:], vp_[:])
                cm5b = csb.tile([NTAG, NCORE], F32, tag="cm5b")
                nc.vector.tensor_mul(cm5b[:], lastvs[:], cm5[:])
                v4 = vp.tile([NTAG, 1], F32)
                nc.vector.reduce_max(v4[:], cm5b[:], axis=AX.X)

                # --- V4: within-chunk backtrace ---
                vall = vp.tile([NTAG, TBLK], F32)
                vnext = vp.tile([NTAG, 1], F32)
                for b in range(CBLK - 1, -1, -1):
                    for s in range(TBLK - 1, -1, -1):
                        t = b * TBLK + s
                        if t == TC - 1:
                            nc.vector.tensor_copy(vall[:, s:s + 1], v4[:])
                        else:
                            vb = cps.tile([NTAG, 1], F32, tag="vps")
                            nc.tensor.matmul(
                                vb[:], g_all[:, (t + 1) * NTAG:(t + 2) * NTAG],
                                vall[:, s + 1:s + 2] if s + 1 < TBLK else vnext[:],
                                start=True, stop=True)
                            nc.vector.tensor_copy(vall[:, s:s + 1], vb[:])
                    # emit tags for this block
                    tgp = cps.tile([TBLK, 1], F32, tag="vps")
                    nc.tensor.matmul(tgp[:], vall[:], iota5[:],
                                     start=True, stop=True)
                    tgi = csb.tile([TBLK, 1], I32, tag="tgi")
                    nc.vector.tensor_copy(tgi[:], tgp[:])
                    nc.sync.dma_start(path_out[b, :], tgi[:])
                    if b > 0:
                        nc.vector.tensor_copy(vnext[:], vall[:, 0:1])

    nc.compile()
    return nc


# ===================== host-side prep =====================

def prep_inputs(inputs, T, TBLK):
    TC = T // NCORE
    CBLK = TC // TBLK
    sentence = np.asarray(inputs["sentence"]).astype(np.int64)
    emb = np.asarray(inputs["emb"], dtype=np.float32)
    x = emb[sentence]                      # [T, EMB]
    x_rev = x[::-1]
    perm = _gate_perm()

    def wih_tiles(W):
        Wp = np.asarray(W, np.float32)[perm]         # [2048, EMB]
        t = np.zeros((NKE, NM, 128, 128), np.float32)
        for k in range(NKE):
            for m in range(NM):
                t[k, m] = Wp[m * 128:(m + 1) * 128, k * 128:(k + 1) * 128].T
        return t

    def whh_tiles(W):
        Wp = np.asarray(W, np.float32)[perm]         # [2048, H2]
        t = np.zeros((NM * NK, 128, 128), np.float32)
        for m in range(NM):
            for k in range(NK):
                t[m * NK + k] = Wp[m * 128:(m + 1) * 128,
                                   k * 128:(k + 1) * 128].T
        return t

    wihT = np.stack([wih_tiles(inputs["W_ih_f"]), wih_tiles(inputs["W_ih_b"])])
    bias = np.zeros((128, 2 * NM), np.float32)
    for d, bkey in enumerate(["b_f", "b_b"]):
        bp = np.asarray(inputs[bkey], np.float32)[perm]
        bias[:, d * NM:(d + 1) * NM] = bp.reshape(NM, 128).T
    whh_f = whh_tiles(inputs["W_hh_f"])
    whh_b = whh_tiles(inputs["W_hh_b"])
    whh_z = np.zeros_like(whh_f)

    h0 = np.asarray(inputs["h0"], np.float32)
    c0 = np.asarray(inputs["c0"], np.float32)

    def h0c0(d):
        out = np.zeros((128, 8), np.float32)
        out[:, 0:4] = h0[d, 0].reshape(4, 128).T
        out[:, 4:8] = c0[d, 0].reshape(4, 128).T
        return out

    W_out = np.asarray(inputs["W_out"], np.float32)      # [NTAG, HID]
    woutT = np.zeros((NKE, 128, NTAG), np.float32)
    for k in range(NKE):
        woutT[k] = W_out[:, k * 128:(k + 1) * 128].T
    bout = np.asarray(inputs["b_out"], np.float32).reshape(1, NTAG)
    trans = np.asarray(inputs["transitions"], np.float32)
    transrep = np.tile(trans.reshape(1, NTAG * NTAG), (NTAG, 1))
    tstop = trans[STOP].reshape(1, NTAG)
    dinit = np.full((NTAG, NTAG), NEG, np.float32)
    np.fill_diagonal(dinit, 0.0)
    fv0 = np.full((1, NTAG), NEG, np.float32)
    fv0[0, START] = 0.0
    iota5 = np.arange(NTAG, dtype=np.float32).reshape(NTAG, 1)

    in_maps = []
    for cidx in range(NCORE):
        xT = np.zeros((2, NKE, 128, TC), np.float32)
        xs_f = x[cidx * TC:(cidx + 1) * TC]
        xs_b = x_rev[cidx * TC:(cidx + 1) * TC]
        for k in range(NKE):
            xT[0, k] = xs_f[:, k * 128:(k + 1) * 128].T
            xT[1, k] = xs_b[:, k * 128:(k + 1) * 128].T
        masks = np.zeros((128, 2), np.float32)
        if cidx == 0:
            masks[:, 0] = 1.0
        if cidx == 1:
            masks[:, 1] = 1.0
        cmrow = np.zeros((1, NCORE * NTAG), np.float32)
        cmrow[0, cidx * NTAG:(cidx + 1) * NTAG] = 1.0
        cm5 = np.zeros((NTAG, NCORE), np.float32)
        cm5[:, cidx] = 1.0
        whh = whh_f if cidx == 0 else (whh_b if cidx == 1 else whh_z)
        hc = h0c0(0) if cidx == 0 else (h0c0(1) if cidx == 1
                                        else np.zeros((128, 8), np.float32))
        in_maps.append(dict(
            xT_in=xT, wihT_in=wihT, bias_in=bias, whh_in=whh, h0c0_in=hc,
            masks_in=masks, woutT_in=woutT, bout_in=bout, trans_in=trans,
            transrep_in=transrep, transstop_in=tstop, dinit_in=dinit,
            fv0_in=fv0, iota5_in=iota5, cmaskrow_in=cmrow, cmask5_in=cm5))
    return in_maps


def assemble_outputs(results, T, TBLK):
    TC = T // NCORE
    score = np.float32(results[0]["score_out"][0, 0])
    path = np.concatenate(
        [results[c]["path_out"].reshape(TC) for c in range(NCORE)])
    return score, path.astype(np.int32)


_PROG_CACHE = {}


def kernel(**inputs):
    T, TBLK = 4096, 128
    key = (T, TBLK)
    if key not in _PROG_CACHE:
        _PROG_CACHE[key] = build_program(T, TBLK)
    nc = _PROG_CACHE[key]
    in_maps = prep_inputs(inputs, T, TBLK)
    res = run_bass_kernel_spmd(nc, in_maps, list(range(NCORE))).results
    return assemble_outputs(res, T, TBLK)
